# revision 7
# baseline (speedup 1.0000x reference)
"""Trainium2 Bass kernel for the Matrix-Tree (HSumGraph) layer.

Strategy: pure data parallel, B=64 -> 8 matrices per core on 8 cores.
Device works in a "rolled" token order (roll -1) so the root row f lands
at index N-1; host rolls outputs back. Inverse of the 512x512 L_bar via
block Gauss-Jordan over 128x128 blocks; pivots 0..2 are column-diag-
dominant -> Newton-Schulz with X0=diag^-1; pivot 3 (contains f-row) via
a bordered 127+1 split with a scalar Schur complement. All fp32.

Outputs on device are dxT (transposed) per matrix; host transposes and
rolls back. Self-contained: hardcodes all shapes.
"""

import numpy as np

import concourse.bacc as bacc
import concourse.mybir as mybir
from concourse.tile import TileContext
from concourse.bass import MemorySpace
from concourse.bass_utils import run_bass_kernel_spmd

F32 = mybir.dt.float32
P = 128
N = 512
H = 300
BPC = 8          # batches per core
NCORES = 8
EPS = 1e-6
KCH = [(0, 128), (128, 128), (256, 44)]   # H=300 chunks
NCH = 4                                    # N=512 -> 4 chunks of 128
NS_ITERS = [10, 12, 14, 22]

_cached = {}


def _build_nc():
    nc = bacc.Bacc(None)
    d_svt = nc.dram_tensor("svt", [BPC, H, N], F32, kind="ExternalInput")
    d_m = nc.dram_tensor("m", [BPC, N], F32, kind="ExternalInput")
    d_wtp = nc.dram_tensor("wtp", [H, H], F32, kind="ExternalInput")
    d_wtc = nc.dram_tensor("wtc", [H, H], F32, kind="ExternalInput")
    d_wbil = nc.dram_tensor("wbil", [H, H], F32, kind="ExternalInput")
    d_btp = nc.dram_tensor("btp", [H, 1], F32, kind="ExternalInput")
    d_btc = nc.dram_tensor("btc", [H, 1], F32, kind="ExternalInput")
    d_wfi = nc.dram_tensor("wfi", [H, 1], F32, kind="ExternalInput")
    d_eye = nc.dram_tensor("eye", [P, P], F32, kind="ExternalInput")
    d_dxt = nc.dram_tensor("dxt", [BPC, N, N], F32, kind="ExternalOutput")
    d_d0 = nc.dram_tensor("d0", [BPC, N], F32, kind="ExternalOutput")

    with TileContext(nc) as tc:
        with (
            tc.tile_pool(name="consts", bufs=1) as consts,
            tc.tile_pool(name="wpool", bufs=1) as wpool,
            tc.tile_pool(name="sb", bufs=2) as sb,
            tc.tile_pool(name="sbk", bufs=3) as sbk,
            tc.tile_pool(name="psb", bufs=3, space=MemorySpace.PSUM) as psb,
            tc.tile_pool(name="pss", bufs=4, space=MemorySpace.PSUM) as pss,
        ):
            ident = consts.tile([P, P], F32)
            nc.sync.dma_start(ident, d_eye[:, :])
            twoI = consts.tile([P, P], F32)
            nc.vector.tensor_scalar_mul(twoI, ident, 2.0)
            ones_row = consts.tile([1, P], F32)
            nc.vector.memset(ones_row, 1.0)
            ones_col = consts.tile([P, 1], F32)
            nc.vector.memset(ones_col, 1.0)
            one_minus_I = consts.tile([P, P], F32)
            nc.vector.tensor_scalar(
                out=one_minus_I, in0=ident, scalar1=-1.0, scalar2=1.0,
                op0=mybir.AluOpType.mult, op1=mybir.AluOpType.add)
            # 1 everywhere except 0 at partition 127
            rm127 = consts.tile([P, 1], F32)
            nc.vector.tensor_scalar(
                out=rm127, in0=ident[:, 127:128], scalar1=-1.0, scalar2=1.0,
                op0=mybir.AluOpType.mult, op1=mybir.AluOpType.add)
            e127 = consts.tile([P, 1], F32)
            nc.vector.tensor_copy(e127, ident[:, 127:128])

            wtp = wpool.tile([P, 3, H], F32)
            wtc = wpool.tile([P, 3, H], F32)
            wbil = wpool.tile([P, 3, H], F32)
            btp = wpool.tile([P, 3, 1], F32)
            btc = wpool.tile([P, 3, 1], F32)
            wfi = wpool.tile([P, 3, 1], F32)
            for kc, (k0, kn) in enumerate(KCH):
                nc.sync.dma_start(wtp[0:kn, kc, :], d_wtp[k0:k0 + kn, :])
                nc.sync.dma_start(wtc[0:kn, kc, :], d_wtc[k0:k0 + kn, :])
                nc.sync.dma_start(wbil[0:kn, kc, :], d_wbil[k0:k0 + kn, :])
                nc.sync.dma_start(btp[0:kn, kc, :], d_btp[k0:k0 + kn, :])
                nc.sync.dma_start(btc[0:kn, kc, :], d_btc[k0:k0 + kn, :])
                nc.sync.dma_start(wfi[0:kn, kc, :], d_wfi[k0:k0 + kn, :])

            for b in range(BPC):
                # ---- load svT, mask ----
                svt = sbk.tile([P, 3, N], F32)
                for kc, (k0, kn) in enumerate(KCH):
                    nc.sync.dma_start(svt[0:kn, kc, :], d_svt[b, k0:k0 + kn, :])
                m_sb = sbk.tile([1, N], F32)
                nc.sync.dma_start(m_sb, d_m[b:b + 1, :])
                mrep_ps = psb.tile([P, N], F32, tag="big")
                nc.tensor.matmul(mrep_ps, ones_row, m_sb, start=True, stop=True)
                m_rep = sbk.tile([P, N], F32)
                nc.vector.tensor_copy(m_rep, mrep_ps)

                # ---- tpT/tcT = relu(W^T x + b)*m  [128,3,N] ----
                tpT = sbk.tile([P, 3, N], F32)
                tcT = sbk.tile([P, 3, N], F32)
                for (W, bias, out) in ((wtp, btp, tpT), (wtc, btc, tcT)):
                    for hc, (h0, hn) in enumerate(KCH):
                        ps = psb.tile([P, N], F32, tag="big")
                        for kc, (k0, kn) in enumerate(KCH):
                            nc.tensor.matmul(
                                ps[0:hn, :], W[0:kn, kc, h0:h0 + hn],
                                svt[0:kn, kc, :],
                                start=(kc == 0), stop=(kc == 2))
                        nc.scalar.activation(
                            out[0:hn, hc, :], ps[0:hn, :],
                            mybir.ActivationFunctionType.Relu,
                            bias=bias[0:hn, hc, :])
                        nc.vector.tensor_mul(
                            out[0:hn, hc, :], out[0:hn, hc, :], m_rep[0:hn, :])

                # ---- uT = Wbil^T tpT ----
                uT = sbk.tile([P, 3, N], F32)
                for ec, (e0, en) in enumerate(KCH):
                    ps = psb.tile([P, N], F32, tag="big")
                    for kc, (k0, kn) in enumerate(KCH):
                        nc.tensor.matmul(
                            ps[0:en, :], wbil[0:kn, kc, e0:e0 + en],
                            tpT[0:kn, kc, :], start=(kc == 0), stop=(kc == 2))
                    nc.vector.tensor_copy(uT[0:en, ec, :], ps[0:en, :])

                # ---- A = relu(scores)+EPS, zero diag  [128,4,N] ----
                A = sbk.tile([P, 4, N], F32)
                for c in range(NCH):
                    ps = psb.tile([P, N], F32, tag="big")
                    for ec, (e0, en) in enumerate(KCH):
                        nc.tensor.matmul(
                            ps, uT[0:en, ec, c * P:(c + 1) * P],
                            tcT[0:en, ec, :], start=(ec == 0), stop=(ec == 2))
                    nc.vector.tensor_scalar(
                        out=A[:, c, :], in0=ps, scalar1=0.0, scalar2=EPS,
                        op0=mybir.AluOpType.max, op1=mybir.AluOpType.add)
                    nc.vector.tensor_mul(
                        A[:, c, c * P:(c + 1) * P],
                        A[:, c, c * P:(c + 1) * P], one_minus_I)

                # ---- AT (negated) for the epilogue ----
                ATn = sbk.tile([P, 4, N], F32)
                for i in range(NCH):
                    for j in range(NCH):
                        tp_ps = pss.tile([P, P], F32, tag="small")
                        nc.tensor.transpose(
                            tp_ps, A[:, i, j * P:(j + 1) * P], ident)
                        nc.vector.tensor_scalar_mul(
                            ATn[:, j, i * P:(i + 1) * P], tp_ps, -1.0)

                # ---- f = relu(root)+EPS ----
                root_ps = pss.tile([1, N], F32, tag="small")
                for kc, (k0, kn) in enumerate(KCH):
                    nc.tensor.matmul(
                        root_ps, wfi[0:kn, kc, :], tpT[0:kn, kc, :],
                        start=(kc == 0), stop=(kc == 2))
                f_row = sbk.tile([1, N], F32)
                nc.vector.tensor_scalar(
                    out=f_row, in0=root_ps, scalar1=0.0, scalar2=EPS,
                    op0=mybir.AluOpType.max, op1=mybir.AluOpType.add)

                # ---- colsum, M = -A + diag(colsum); row 511 <- f ----
                cs_ps = pss.tile([1, N], F32, tag="small")
                for c in range(NCH):
                    nc.tensor.matmul(cs_ps, ones_col, A[:, c, :],
                                     start=(c == 0), stop=(c == 3))
                cs_sb = sbk.tile([1, N], F32)
                nc.vector.tensor_copy(cs_sb, cs_ps)
                csr_ps = psb.tile([P, N], F32, tag="big")
                nc.tensor.matmul(csr_ps, ones_row, cs_sb, start=True, stop=True)
                cs_rep = sbk.tile([P, N], F32)
                nc.vector.tensor_copy(cs_rep, csr_ps)

                M = sbk.tile([P, 4, N], F32)
                nc.vector.tensor_scalar_mul(M, A, -1.0)
                for c in range(NCH):
                    nc.vector.copy_predicated(
                        M[:, c, c * P:(c + 1) * P],
                        ident.bitcast(mybir.dt.uint32),
                        cs_rep[:, c * P:(c + 1) * P])
                nc.sync.dma_start(M[127:128, 3, :], f_row)

                # ---- block Gauss-Jordan ----
                for k in range(4):
                    kcols = slice(k * P, (k + 1) * P)
                    pt_ps = pss.tile([P, P], F32, tag="small")
                    nc.tensor.transpose(pt_ps, M[:, k, kcols], ident)
                    PT = sb.tile([P, P], F32, tag="PT")
                    nc.vector.tensor_copy(PT, pt_ps)

                    if k < 3:
                        piv = M[:, k, kcols]
                        pivT = PT
                    else:
                        piv = sb.tile([P, P], F32, tag="spad")
                        nc.vector.tensor_scalar_mul(piv, M[:, k, kcols], rm127)
                        nc.vector.memset(piv[:, 127:128], 0.0)
                        nc.vector.tensor_add(
                            piv[:, 127:128], piv[:, 127:128], e127)
                        pivT = sb.tile([P, P], F32, tag="spadT")
                        nc.vector.tensor_scalar_mul(pivT, PT, rm127)
                        nc.vector.memset(pivT[:, 127:128], 0.0)
                        nc.vector.tensor_add(
                            pivT[:, 127:128], pivT[:, 127:128], e127)

                    # Newton-Schulz with X0 = diag(piv)^-1
                    dtmp = sb.tile([P, P], F32, tag="dtmp")
                    nc.vector.tensor_mul(dtmp, piv, ident)
                    dcol = sb.tile([P, 1], F32, tag="dcol")
                    nc.vector.tensor_reduce(
                        dcol, dtmp, mybir.AxisListType.X, mybir.AluOpType.add)
                    rcol = sb.tile([P, 1], F32, tag="rcol")
                    nc.vector.reciprocal(rcol, dcol)
                    X = sb.tile([P, P], F32, tag="X")
                    nc.vector.tensor_scalar_mul(X, ident, rcol)
                    XT = sb.tile([P, P], F32, tag="XT")
                    nc.vector.tensor_copy(XT, X)

                    for it in range(NS_ITERS[k]):
                        t_ps = pss.tile([P, P], F32, tag="small")
                        nc.tensor.matmul(t_ps, pivT, X, start=True, stop=True)
                        E = sb.tile([P, P], F32, tag="E")
                        nc.vector.tensor_sub(E, twoI, t_ps)
                        xn_ps = pss.tile([P, P], F32, tag="small")
                        nc.tensor.matmul(xn_ps, XT, E, start=True, stop=True)
                        X = sb.tile([P, P], F32, tag="X")
                        nc.scalar.copy(X, xn_ps)
                        xt_ps = pss.tile([P, P], F32, tag="small")
                        nc.tensor.transpose(xt_ps, X, ident)
                        XT = sb.tile([P, P], F32, tag="XT")
                        nc.vector.tensor_copy(XT, xt_ps)

                    if k < 3:
                        Pinv, PinvT = X, XT
                    else:
                        # bordered 127+1 assembly
                        w_ps = pss.tile([P, 1], F32, tag="small")
                        nc.tensor.matmul(w_ps, XT, M[:, 3, 511:512],
                                         start=True, stop=True)
                        w_sb = sb.tile([P, 1], F32, tag="w_sb")
                        nc.vector.tensor_scalar_mul(w_sb, w_ps, rm127)
                        nc.vector.tensor_sub(w_sb, w_sb, e127)
                        fr_sb = sb.tile([P, 1], F32, tag="fr_sb")
                        nc.vector.tensor_copy(fr_sb, PT[:, 127:128])
                        q_ps = pss.tile([P, 1], F32, tag="small")
                        fr2 = sb.tile([P, 1], F32, tag="fr2")
                        nc.vector.tensor_scalar_mul(fr2, fr_sb, rm127)
                        nc.tensor.matmul(q_ps, X, fr2, start=True, stop=True)
                        q_sb = sb.tile([P, 1], F32, tag="q_sb")
                        nc.vector.tensor_scalar_mul(q_sb, q_ps, rm127)
                        nc.vector.tensor_sub(q_sb, q_sb, e127)
                        dot_ps = pss.tile([1, 1], F32, tag="small")
                        nc.tensor.matmul(dot_ps, fr_sb, w_sb,
                                         start=True, stop=True)
                        si_sb = sb.tile([1, 1], F32, tag="si_sb")
                        nc.vector.tensor_scalar_mul(si_sb, dot_ps, -1.0)
                        nc.vector.reciprocal(si_sb, si_sb)
                        sir_ps = pss.tile([P, 1], F32, tag="small")
                        nc.tensor.matmul(sir_ps, ones_row, si_sb,
                                         start=True, stop=True)
                        si_col = sb.tile([P, 1], F32, tag="si_col")
                        nc.vector.tensor_copy(si_col, sir_ps)
                        wr_ps = pss.tile([1, P], F32, tag="small")
                        nc.tensor.matmul(wr_ps, w_sb, ident,
                                         start=True, stop=True)
                        w_row = sb.tile([1, P], F32, tag="w_row")
                        nc.vector.tensor_copy(w_row, wr_ps)
                        qr_ps = pss.tile([1, P], F32, tag="small")
                        nc.tensor.matmul(qr_ps, q_sb, ident,
                                         start=True, stop=True)
                        q_row = sb.tile([1, P], F32, tag="q_row")
                        nc.vector.tensor_copy(q_row, qr_ps)
                        op_ps = pss.tile([P, P], F32, tag="small")
                        nc.tensor.matmul(op_ps, w_row, q_row,
                                         start=True, stop=True)
                        Xz = sb.tile([P, P], F32, tag="Xz")
                        nc.vector.tensor_scalar_mul(Xz, X, rm127)
                        Pinv = sb.tile([P, P], F32, tag="X")
                        nc.vector.scalar_tensor_tensor(
                            out=Pinv, in0=op_ps, scalar=si_col, in1=Xz,
                            op0=mybir.AluOpType.mult, op1=mybir.AluOpType.add)
                        opt_ps = pss.tile([P, P], F32, tag="small")
                        nc.tensor.matmul(opt_ps, q_row, w_row,
                                         start=True, stop=True)
                        XTz = sb.tile([P, P], F32, tag="XTz")
                        nc.vector.tensor_scalar_mul(XTz, XT, rm127)
                        PinvT = sb.tile([P, P], F32, tag="XT")
                        nc.vector.scalar_tensor_tensor(
                            out=PinvT, in0=opt_ps, scalar=si_col, in1=XTz,
                            op0=mybir.AluOpType.mult, op1=mybir.AluOpType.add)

                    # row k: R = Pinv @ M[k, :]
                    r_ps = psb.tile([P, N], F32, tag="big")
                    nc.tensor.matmul(r_ps, PinvT, M[:, k, :],
                                     start=True, stop=True)
                    # other rows: save CiT, zero col block, update
                    ciTs = []
                    for i in range(4):
                        if i == k:
                            continue
                        cit_ps = pss.tile([P, P], F32, tag="small")
                        nc.tensor.transpose(cit_ps, M[:, i, kcols], ident)
                        CiT = sb.tile([P, P], F32, tag=f"CiT{len(ciTs)}")
                        nc.vector.tensor_copy(CiT, cit_ps)
                        ciTs.append((i, CiT))
                    nc.vector.tensor_copy(M[:, k, :], r_ps)
                    nc.scalar.copy(M[:, k, kcols], Pinv)
                    for i, CiT in ciTs:
                        nc.vector.memset(M[:, i, kcols], 0.0)
                        u_ps = psb.tile([P, N], F32, tag="big")
                        nc.tensor.matmul(u_ps, CiT, M[:, k, :],
                                         start=True, stop=True)
                        nc.vector.tensor_sub(M[:, i, :], M[:, i, :], u_ps)

                # ---- epilogue: d0, dxT ----
                d0_col = sbk.tile([P, 4], F32)
                for c in range(NCH):
                    ft_ps = pss.tile([P, 1], F32, tag="small")
                    nc.tensor.matmul(
                        ft_ps, f_row[0:1, c * P:(c + 1) * P],
                        ones_row[0:1, 0:1], start=True, stop=True)
                    nc.vector.tensor_mul(
                        d0_col[:, c:c + 1], ft_ps, M[:, c, 511:512])
                nc.sync.dma_start(
                    d_d0[b].rearrange("(c p) -> p c", p=P), d0_col)

                # diag of V
                dg4 = sbk.tile([P, 4], F32)
                for c in range(NCH):
                    dtmp2 = sb.tile([P, P], F32, tag="dtmp2")
                    nc.vector.tensor_mul(
                        dtmp2, M[:, c, c * P:(c + 1) * P], ident)
                    nc.vector.tensor_reduce(
                        dg4[:, c:c + 1], dtmp2,
                        mybir.AxisListType.X, mybir.AluOpType.add)
                nc.vector.tensor_scalar_mul(dg4[:, 3:4], dg4[:, 3:4], rm127)
                nc.vector.memset(M[:, :, 511:512], 0.0)
                for c in range(NCH):
                    vm = sb.tile([P, N], F32, tag="vm")
                    nc.vector.tensor_scalar(
                        out=vm, in0=M[:, c, :], scalar1=dg4[:, c:c + 1],
                        scalar2=None, op0=mybir.AluOpType.subtract)
                    nc.vector.tensor_mul(ATn[:, c, :], ATn[:, c, :], vm)
                    nc.sync.dma_start(
                        d_dxt[b, c * P:(c + 1) * P, :], ATn[:, c, :])
    nc.compile()
    return nc


def _get_nc():
    if "nc" not in _cached:
        _cached["nc"] = _build_nc()
    return _cached["nc"]


def _run(inputs, trace=False):
    sv = np.ascontiguousarray(inputs["sent_vecs"], dtype=np.float32)
    msk = np.ascontiguousarray(
        inputs["enc_sent_padding_mask"], dtype=np.float32)
    B = sv.shape[0]
    svr = np.roll(sv, -1, axis=1)                    # rolled token order
    mr = np.roll(msk, -1, axis=1)
    svt = np.ascontiguousarray(svr.transpose(0, 2, 1))  # [B, H, N]

    wtp = np.ascontiguousarray(inputs["Wtp"], dtype=np.float32)
    wtc = np.ascontiguousarray(inputs["Wtc"], dtype=np.float32)
    wbil = np.ascontiguousarray(inputs["Wbil"], dtype=np.float32)
    btp = np.ascontiguousarray(inputs["btp"], dtype=np.float32).reshape(H, 1)
    btc = np.ascontiguousarray(inputs["btc"], dtype=np.float32).reshape(H, 1)
    wfi = np.ascontiguousarray(inputs["wfi"], dtype=np.float32).reshape(H, 1)
    eye = np.eye(P, dtype=np.float32)

    nc = _get_nc()
    in_maps = []
    for c in range(NCORES):
        sl = slice(c * BPC, (c + 1) * BPC)
        in_maps.append({
            "svt": np.ascontiguousarray(svt[sl]),
            "m": np.ascontiguousarray(mr[sl]),
            "wtp": wtp, "wtc": wtc, "wbil": wbil,
            "btp": btp, "btc": btc, "wfi": wfi, "eye": eye,
        })
    res = run_bass_kernel_spmd(
        nc, in_maps, core_ids=list(range(NCORES)), trace=trace)

    dxt = np.concatenate([res.results[c]["dxt"] for c in range(NCORES)], 0)
    d0r = np.concatenate([res.results[c]["d0"] for c in range(NCORES)], 0)
    dx_dev = dxt.transpose(0, 2, 1)                  # un-transpose
    dx = np.roll(np.roll(dx_dev, 1, axis=1), 1, axis=2)  # un-roll
    d0 = np.roll(d0r, 1, axis=1)
    return (np.ascontiguousarray(dx), np.ascontiguousarray(d0)), res


def kernel(**inputs):
    (dx, d0), _ = _run(inputs, trace=False)
    return dx, d0
